# revision 70
# baseline (speedup 1.0000x reference)
"""Trainium2 Bass kernel for nn_ContextKGEModel (self-attentive path pooling + FFN hinge loss).

Data-parallel over the 2048 ragged groups, 8 NeuronCores:
  - Host: assign 16 whole batch rows per core (load-balanced), pack each
    core's 256 groups into full 128-row bins with a greedy exact-fit packer
    (33 bins, zero padding for the canonical size distribution), and ship
    triple_emb in two fp8-e4m3 layouts (row-major bins with an appended ones
    column, and a d-major transposed copy in supertiles of up to 4 bins).
    The cross-group mask is encoded as a tiny per-bin +/-240 rank-G+1 factor
    pair so the mask is APPLIED BY THE TENSOR ENGINE as one extra accumulation
    matmul into the Gram PSUM (no mask DMA blocks, no vector mask-add).
    Weights are replicated and pre-transposed; W1/b1 host-scaled by 8 and W2
    by 16 so they stay in fp8 normal range (1/128 folds into the sigmoid
    scale). A +/-1 pair-selection matrix encodes the hinge pairs.
  - Device (per core): xwT = W_sfa^T @ X^T per supertile and per-bin Gram
    xw X^T run as fp8 DoubleRow matmuls; the group-masked column max is taken
    on the raw Gram (tanh is monotone so it commutes with max) with the mask
    already accumulated in PSUM; one fused tanh + one exp per supertile;
    attention weights are built ON THE GPSIMD ENGINE by an iota-vs-slot
    compare fused with the exp scale; unnormalized pooled vectors accumulate
    in PSUM across all bins (the ones column accumulates the softmax
    denominator); the FFN runs fp8 DoubleRow with the score reduction as
    DoubleRow pairs; the hinge loss is computed on-chip via the
    pair-selection matmul. PSUM->SBUF evictions alternate between the
    Activation and Vector engines so neither becomes the pipeline
    bottleneck. Host sums the 8 partial losses.
"""

import os
import threading
from contextlib import ExitStack

import numpy as np
import ml_dtypes

import concourse.bass as bass
import concourse.tile as tile
from concourse import mybir
from concourse.vector_clock import ScopedClock
from concourse.bass_utils import run_bass_kernel_spmd
from concourse.masks import make_identity

bf16 = ml_dtypes.bfloat16
fp8e4 = ml_dtypes.float8_e4m3

B, NEG, L, D = 128, 15, 32, 768
NPAIR_SET = 120                      # 240 hinge pairs split into 2 matmul sets
G = B * (NEG + 1)
GAMMA = 0.1
NCORES = 8
ROWS_PER_CORE = B // NCORES          # 16 batch rows / core
SLOTS = ROWS_PER_CORE * (NEG + 1)    # 256 group slots / core
BIN = 128
KC = D // 128                        # 6 contraction chunks
HC = (4 * D) // 128                  # 24 hidden chunks
DW = D + 8                           # x row + ones column + pad
NEG_MASK = -240.0

_compile_cache = {}
_compile_lock = threading.Lock()


def _patch_tile_drain():
    """This walrus build rejects >1 sem-wait on an instruction ("Too many sync
    wait commands"); split the TileContext tail-drain waits across SP nops."""
    if getattr(tile.TileContext, "_drain_patch_applied", False):
        return

    def _drain_and_barrier(self, tick_clock, wait_clock):
        probe = self.nc.sync.nop(nofuse=True, hint="drain_wait_split")
        wait_clock.add_sem_waits(probe.ins, ScopedClock({None: tick_clock.global_clock}))
        si = probe.ins.sync_info
        waits = list(si.on_wait) if si is not None and si.on_wait else []
        if len(waits) > 1:
            si.on_wait = waits[:1]
            for w in waits[1:]:
                extra = self.nc.sync.nop(nofuse=True, hint="drain_wait_split")
                esi = extra.ins.sync_info
                if esi is None:
                    extra.ins.sync_info = mybir.SyncInfo(on_wait=[w], on_update=[])
                else:
                    esi.on_wait = [w]
        self.nc.sync.drain()
        self.nc.all_engine_barrier()
        assert self.sems is not None
        popped = self.nc._tile_sem_poison_stack.pop()
        assert popped is self._sem_poison
        self.nc.clear_and_free_semaphores(list(self.sems.allocated().values()))
        self.nc.all_engine_barrier()

    tile.TileContext._drain_and_barrier = _drain_and_barrier
    tile.TileContext._drain_patch_applied = True


_MAX_WAITS = 1


def _split_waits(nc, maxw=_MAX_WAITS):
    """Hoist excess sync-waits onto NoOps inserted just before the
    instruction on the same engine (walrus build caps waits/instruction)."""
    n_split = 0
    for fn in nc.m.functions:
        for bb in fn.blocks:
            out = []
            for inst in bb.instructions:
                si = inst.sync_info
                waits = list(si.on_wait) if si is not None and si.on_wait else []
                if len(waits) > maxw:
                    keep = waits[:maxw]
                    rest = waits[maxw:]
                    for i in range(0, len(rest), maxw):
                        n_split += 1
                        nop = mybir.InstNoOp(
                            name=f"WSPLIT-{n_split}",
                            engine=inst.engine,
                            debug=inst.debug,
                            ins=[], outs=[],
                            sync_info=mybir.SyncInfo(
                                on_wait=rest[i:i + maxw], on_update=[]),
                        )
                        out.append(nop)
                    si.on_wait = keep
                out.append(inst)
            if n_split:
                bb.instructions[:] = out
    return n_split


# ---------------------------------------------------------------- host packing

def _pack(sizes_flat):
    """Balanced batch-row -> core assignment + greedy exact-fit bin packing
    (full 128-row bins; 33 bins/core for the canonical distribution)."""
    sizes = sizes_flat.reshape(B, NEG + 1)
    row_load = sizes.sum(1)
    order = np.argsort(-row_load, kind="stable")
    core_rows = [[] for _ in range(NCORES)]
    core_load = np.zeros(NCORES, np.int64)
    for b in order:
        cands = [c for c in range(NCORES) if len(core_rows[c]) < ROWS_PER_CORE]
        c = min(cands, key=lambda c: core_load[c])
        core_rows[c].append(int(b))
        core_load[c] += row_load[b]
    bins_all = []
    for c in range(NCORES):
        groups = []
        for lb, b in enumerate(core_rows[c]):
            for k in range(NEG + 1):
                g = b * (NEG + 1) + k
                groups.append((g, lb * (NEG + 1) + k, int(sizes_flat[g])))
        groups.sort(key=lambda t: -t[2])
        remaining = list(groups)
        bins = []
        while remaining:
            cap = BIN
            bn = []
            while cap > 0 and remaining:
                pick = None
                for idx, (g, slot, n) in enumerate(remaining):
                    if n == cap:
                        pick = idx
                        break
                if pick is None:
                    for idx, (g, slot, n) in enumerate(remaining):
                        if n <= cap:
                            pick = idx
                            break
                if pick is None:
                    break
                g, slot, n = remaining.pop(pick)
                bn.append((g, slot, n, BIN - cap))
                cap -= n
            bins.append(bn)
        bins_all.append(bins)
    return core_rows, bins_all


def _widths_of(nbins):
    """Supertile widths: as many 4-bin supertiles as possible + one tail."""
    w = [4] * (nbins // 4)
    if nbins % 4:
        w.append(nbins % 4)
    return tuple(w)


def _build_core_arrays(bins_c, triple_f8, offsets, NB, MG):
    """Per-core packed device inputs."""
    x = np.zeros((128, NB, DW), fp8e4)            # [row, bin, d] row-major + ones
    xt = np.zeros((128, NB, KC, BIN), np.float32)  # [dlane, bin, chunk, row] staging
    m = np.zeros((MG, NB, 2, BIN), fp8e4)          # mask factors M1 / M2
    slot_of = np.full((128, NB), -1.0, np.float32)
    for bi, bn in enumerate(bins_c):
        for qi, (g, slot, n, off) in enumerate(bn):
            rows = triple_f8[offsets[g]:offsets[g] + n]       # [n, D] fp8
            x[off:off + n, bi, :D] = rows
            x[off:off + n, bi, D] = 1.0
            xt_rows = rows.astype(np.float32).reshape(n, KC, 128)
            xt[:, bi, :, off:off + n] = xt_rows.transpose(2, 1, 0)
            slot_of[off:off + n, bi] = float(slot)
            # mask = 480*same - 480: fp8e4 caps at +-240, so the factor of 2
            # rides on the M2 side (240*2); -480 must beat the most negative
            # own-group Gram max (~ -6 sigma = -170) against cross-group cells
            m[qi, bi, 0, off:off + n] = 240.0
            m[qi, bi, 1, off:off + n] = 2.0
        m[MG - 1, bi, 0, :] = -240.0
        m[MG - 1, bi, 1, :] = 2.0
    xt8 = xt.astype(fp8e4)
    widths = _widths_of(NB)
    # xt flat layout: per supertile contiguous [dlane, chunk, w, row]
    blocks = []
    b0 = 0
    for w in widths:
        blk = xt8[:, b0:b0 + w].transpose(0, 2, 1, 3).reshape(128, KC * w * BIN)
        blocks.append(blk)
        b0 += w
    xt_flat = np.ascontiguousarray(np.concatenate(blocks, axis=1))
    return np.ascontiguousarray(x), xt_flat, np.ascontiguousarray(m), \
        np.ascontiguousarray(slot_of)


# ---------------------------------------------------------------- device program

def _build_program(widths, MG, unit_halves):
    NB = sum(widths)
    NST = len(widths)
    # pooled accumulation units: (st, bin_pair_or_single, local bins)
    units = []
    for s, w in enumerate(widths):
        for bp in range(w // 2):
            units.append((s, 2 * bp, 2))
        if w % 2:
            units.append((s, w - 1, 1))
    NU = len(units)
    # per-half accumulation chain membership for start/stop flags
    chain = {h: [u for u in range(NU) if h in unit_halves[u]] for h in (0, 1)}

    nc = bass.Bass()
    dt = mybir.dt
    AF = mybir.ActivationFunctionType

    x_d = nc.dram_tensor("x_bins", [128, NB, DW], dt.float8e4, kind="ExternalInput")
    xt_d = nc.dram_tensor("xt_bins", [128, KC * NB * BIN], dt.float8e4,
                          kind="ExternalInput")
    m_d = nc.dram_tensor("mask_f", [MG, NB, 2, BIN], dt.float8e4,
                         kind="ExternalInput")
    slot_d = nc.dram_tensor("slot_of", [128, NB], dt.float32, kind="ExternalInput")
    wsfa_d = nc.dram_tensor("w_sfa_t", [128, KC * D], dt.float8e4, kind="ExternalInput")
    w1t_d = nc.dram_tensor("w1_t", [128, KC * 4 * D], dt.float8e4, kind="ExternalInput")
    w2t_d = nc.dram_tensor("w2_t", [128, HC], dt.float8e4, kind="ExternalInput")
    w1b_d = nc.dram_tensor("w1_b", [1, 4 * D], dt.float8e4, kind="ExternalInput")
    b2_d = nc.dram_tensor("b2_r", [128, 1], dt.float32, kind="ExternalInput")
    pair_d = nc.dram_tensor("pair_m", [128, 2, 2, NPAIR_SET], dt.float32,
                            kind="ExternalInput")
    loss_d = nc.dram_tensor("loss", [1, 1], dt.float32, kind="ExternalOutput")

    st_off = []      # column offset of each supertile in xt_d / bin index base
    b0 = 0
    for w in widths:
        st_off.append(b0)
        b0 += w

    with tile.TileContext(nc) as tc, ExitStack() as ctx:
        consts = ctx.enter_context(tc.tile_pool(name="consts", bufs=1))
        xres = ctx.enter_context(tc.tile_pool(name="xres", bufs=1))
        attres = ctx.enter_context(tc.tile_pool(name="attres", bufs=1))
        xt_pool = ctx.enter_context(tc.tile_pool(name="xt", bufs=4))
        xwt_pool = ctx.enter_context(tc.tile_pool(name="xwt", bufs=4))
        small = ctx.enter_context(tc.tile_pool(name="small", bufs=12))
        cm_pool = ctx.enter_context(tc.tile_pool(name="cm", bufs=8))
        ffn_pool = ctx.enter_context(tc.tile_pool(name="ffn", bufs=1))

        # resident constants (wsfa + first supertile loads issued first so
        # compute starts as early as the serial DMA stream allows; wsfa comes
        # in 3 separately-tracked k-pair tiles so the first xw matmul only
        # waits on the first part)
        wsfa_k = [consts.tile([128, 2, D], dt.float8e4, tag=f"wsfa{i}",
                              name=f"wsfa{i}")
                  for i in range(KC // 2)]

        def load_wsfa(i):
            nc.sync.dma_start(
                out=wsfa_k[i],
                in_=wsfa_d[:, 2 * i * D:2 * (i + 1) * D].rearrange(
                    "p (k e) -> p k e", k=2))

        load_wsfa(0)

        x_tiles = [xres.tile([128, widths[s], DW], dt.float8e4, tag=f"x{s}",
                             name=f"x{s}") for s in range(NST)]
        # half-pure units only need a 128-wide attention window
        att_tiles = [attres.tile([128, nb, 128 * len(unit_halves[u])],
                                 dt.float8e4, tag=f"a{u}", name=f"a{u}")
                     for u, (_, _, nb) in enumerate(units)]

        xt_tiles = {}

        def load_xt(s):
            w = widths[s]
            xt_t = xt_pool.tile([128, KC, w * BIN], dt.float8e4, tag="xt",
                                name=f"xt{s}")
            off = KC * st_off[s] * BIN
            nc.sync.dma_start(
                out=xt_t,
                in_=xt_d[:, off:off + KC * w * BIN].rearrange(
                    "p (k c) -> p k c", k=KC))
            xt_tiles[s] = xt_t

        def load_x(s):
            # row-major x is only consumed by pooled (3 supertiles behind), so
            # its loads trail the xt stream instead of clogging the ramp
            w = widths[s]
            nc.sync.dma_start(out=x_tiles[s], in_=x_d[:, st_off[s]:st_off[s] + w, :])

        load_xt(0)
        load_wsfa(1)
        load_wsfa(2)
        load_xt(1)
        m_all = consts.tile([MG, NB, 2, BIN], dt.float8e4)
        nc.sync.dma_start(out=m_all, in_=m_d[:, :, :, :])
        slot_all = consts.tile([128, NB], dt.float32)
        nc.sync.dma_start(out=slot_all, in_=slot_d[:, :])
        ident = consts.tile([128, 128], dt.bfloat16)
        make_identity(nc, ident)
        # w1t carries 2 extra contraction chunks: chunk KC row 0 = b1*8 (pairs
        # with pooledT's ones row), chunk KC+1 = zero DoubleRow padding. The
        # zero regions are memset on the idle Pool engine during the DMA ramp;
        # the weight payloads stream in late (after the phase-A loads).
        w1t = consts.tile([128, KC + 2, 4 * D], dt.float8e4)
        nc.gpsimd.memset(w1t[:, KC:KC + 2, :], 0.0)
        iota_i = consts.tile([128, SLOTS], dt.int32)
        nc.gpsimd.iota(iota_i, pattern=[[1, SLOTS]], base=0, channel_multiplier=0)
        iota_f = consts.tile([128, SLOTS], dt.float32)
        nc.vector.tensor_copy(iota_f, iota_i)

        # ---- phase A: xwT per supertile; per-bin Gram+mask, fused tanh/exp,
        # gpsimd att build one supertile behind; pooled accumulation two
        # supertiles behind
        with (
            tc.tile_pool(name="ps_xw", bufs=3, space="PSUM") as ps_xw,
            tc.tile_pool(name="ps_gm", bufs=1, space="PSUM") as ps_gm,
            tc.tile_pool(name="ps_pool", bufs=1, space="PSUM") as ps_pooled,
        ):
            xwt_tiles = {}
            evict_flip = [0]

            def emit_xw(s):
                w = widths[s]
                xt_t = xt_tiles[s]
                xwt_t = xwt_pool.tile([128, KC, w * BIN], dt.float8e4,
                                      tag="xwt", name=f"xwt{s}")
                for e in range(KC):
                    ps = ps_xw.tile([128, 4 * BIN], dt.float32, tag="psxw",
                                    name=f"psxw{s}_{e}")
                    for k in range(0, KC, 2):
                        nc.tensor.matmul(
                            ps[:, :w * BIN],
                            wsfa_k[k // 2][:, :, e * 128:(e + 1) * 128],
                            xt_t[:, k:k + 2, :],
                            start=(k == 0), stop=(k == KC - 2),
                            perf_mode=mybir.MatmulPerfMode.DoubleRow)
                    if evict_flip[0] % 2 == 0:
                        nc.scalar.copy(xwt_t[:, e, :], ps[:, :w * BIN])
                    else:
                        nc.vector.tensor_copy(xwt_t[:, e, :], ps[:, :w * BIN])
                    evict_flip[0] += 1
                xwt_tiles[s] = xwt_t

            unit_base = {}
            ub = 0
            for s, w in enumerate(widths):
                unit_base[s] = ub
                ub += (w // 2) + (w % 2)

            def emit_bins(s):
                w = widths[s]
                xt_t, xwt_t = xt_tiles[s], xwt_tiles[s]
                ps_g = ps_gm.tile([128, 4, BIN], dt.float32, tag="psgm",
                                  name=f"psgm{s}")
                for lb in range(w):
                    bi = st_off[s] + lb
                    sl = slice(lb * BIN, (lb + 1) * BIN)
                    for e in range(0, KC, 2):
                        nc.tensor.matmul(ps_g[:, lb, :], xwt_t[:, e:e + 2, sl],
                                         xt_t[:, e:e + 2, sl],
                                         start=(e == 0), stop=False,
                                         perf_mode=mybir.MatmulPerfMode.DoubleRow)
                    # cross-group mask as one accumulation matmul:
                    # M1^T M2 = 240*same - 240
                    nc.tensor.matmul(ps_g[:, lb, :], m_all[:, bi, 0, :],
                                     m_all[:, bi, 1, :],
                                     start=False, stop=True)
                # masked max of raw Gram; tanh applied after the max
                # (tanh is monotone, so max commutes with it); latency of this
                # fused chain is hidden by the 3-supertile pooled distance
                cm = cm_pool.tile([128, 4], dt.float32, tag="cm", name=f"cm{s}")
                nc.vector.tensor_reduce(
                    out=cm[:, :w], in_=ps_g[:, :w, :],
                    op=mybir.AluOpType.max, axis=mybir.AxisListType.X)
                th = cm_pool.tile([128, 4], dt.float32, tag="th", name=f"th{s}")
                nc.scalar.activation(th[:, :w], cm[:, :w], AF.Tanh)
                ex = cm_pool.tile([128, 4], dt.float32, tag="ex", name=f"ex{s}")
                nc.scalar.activation(ex[:, :w], th[:, :w], AF.Exp)
                nunits = (w // 2) + (w % 2)
                # last supertiles' att on DVE: Pool's serial backlog would
                # otherwise gate the final pooled accumulations
                att_eng = nc.vector if s >= NST - 2 else nc.gpsimd
                for ui in range(nunits):
                    lb0 = 2 * ui
                    nb = 2 if lb0 + 1 < w else 1
                    u = unit_base[s] + ui
                    att_t = att_tiles[u]
                    halves = sorted(unit_halves[u])
                    io_sl = (slice(halves[0] * 128, (halves[0] + 1) * 128)
                             if len(halves) == 1 else slice(0, SLOTS))
                    for j in range(nb):
                        bi = st_off[s] + lb0 + j
                        att_eng.tensor_scalar(
                            out=att_t[:, j, :], in0=iota_f[:, io_sl],
                            scalar1=slot_all[:, bi:bi + 1],
                            scalar2=ex[:, lb0 + j:lb0 + j + 1],
                            op0=mybir.AluOpType.is_equal,
                            op1=mybir.AluOpType.mult)

            ps_p = [ps_pooled.tile([128, DW], dt.float32, tag=f"psp{h}",
                                   name=f"psp{h}") for h in range(2)]

            def emit_pooled(s):
                w = widths[s]
                xv = x_tiles[s]
                nunits = (w // 2) + (w % 2)
                for ui in range(nunits):
                    u = unit_base[s] + ui
                    lb0 = 2 * ui
                    nb = units[u][2]
                    att_t = att_tiles[u]
                    halves = sorted(unit_halves[u])
                    kw = ({"perf_mode": mybir.MatmulPerfMode.DoubleRow}
                          if nb == 2 else {})
                    for h in halves:
                        hsl = (slice(0, 128) if len(halves) == 1
                               else slice(h * 128, (h + 1) * 128))
                        # keep each matmul output inside one PSUM bank
                        for n0, nlen in ((0, 512), (512, DW - 512)):
                            nc.tensor.matmul(
                                ps_p[h][:, n0:n0 + nlen],
                                att_t[:, :, hsl],
                                xv[:, lb0:lb0 + nb, n0:n0 + nlen],
                                start=(u == chain[h][0]),
                                stop=(u == chain[h][-1]), **kw)

            for s in range(NST):
                emit_xw(s)
                if s + 2 < NST:
                    load_xt(s + 2)
                load_x(s)
                if s >= 1:
                    emit_bins(s - 1)
                if s >= 3:
                    emit_pooled(s - 3)
            emit_bins(NST - 1)
            emit_pooled(NST - 3)
            emit_pooled(NST - 2)
            emit_pooled(NST - 1)

            # FFN weights loaded late so they don't delay the phase-A DMA stream
            nc.sync.dma_start(out=w1t[:, :KC, :],
                              in_=w1t_d[:, :].rearrange("p (k h) -> p k h", k=KC))
            nc.sync.dma_start(out=w1t[0:1, KC, :], in_=w1b_d[:, :])
            w2t = consts.tile([128, HC], dt.float8e4)
            nc.sync.dma_start(out=w2t, in_=w2t_d[:, :])
            b2s = consts.tile([128, 1], dt.float32)
            nc.sync.dma_start(out=b2s, in_=b2_d[:, :])
            pairm = consts.tile([128, 2, 2, NPAIR_SET], dt.float32)
            nc.sync.dma_start(out=pairm, in_=pair_d[:, :, :, :])

            # ---- phase B1: normalize pooled by the accumulated denominator
            # (separate tiles per slot-half so each half's transposes only
            # wait on its own normalize)
            pooled_sb = [ffn_pool.tile([128, D], dt.bfloat16, tag=f"pooled{h}",
                                       name=f"pooled{h}") for h in range(2)]
            for h in range(2):
                rz = small.tile([128, 1], dt.float32, tag="rz", name=f"rz{h}")
                nc.vector.reciprocal(rz, ps_p[h][:, D:D + 1])
                if h == 0:
                    nc.scalar.activation(pooled_sb[h], ps_p[h][:, :D],
                                         AF.Copy, scale=rz)
                else:
                    nc.vector.tensor_scalar_mul(pooled_sb[h],
                                                ps_p[h][:, :D], rz)

        # ---- phase B2: transpose pooled, FFN, hinge loss
        with (
            tc.tile_pool(name="ps_t", bufs=3, space="PSUM") as ps_t,
            tc.tile_pool(name="ps_h", bufs=3, space="PSUM") as ps_h,
            tc.tile_pool(name="ps_sc", bufs=1, space="PSUM") as ps_sc,
        ):
            # transpose pooled in batches of 4: one [128, 4*128] PSUM tile per
            # eviction so the PSUM->SBUF copies are few and wide. pooledT has
            # 2 extra contraction chunks: chunk KC carries a ones row at
            # partition 0 (pairs with the b1 row folded into w1t on host) and
            # chunk KC+1 is zero padding for the DoubleRow pair.
            pooledT = ffn_pool.tile([128, KC + 2, SLOTS], dt.float8e4,
                                    tag="pooledT")
            nc.gpsimd.memset(pooledT[:, KC:KC + 2, :], 0.0)
            nc.gpsimd.memset(pooledT[0:1, KC, :], 1.0)
            # h-major so the first transposes only wait on h0's normalize
            tr_jobs = [(k, h) for h in range(2) for k in range(KC)]
            for grp in range(3):
                ps_tr = ps_t.tile([128, 4, 128], dt.bfloat16, tag="pstr",
                                  name=f"pstr{grp}")
                for q in range(4):
                    k, h = tr_jobs[4 * grp + q]
                    nc.tensor.transpose(
                        ps_tr[:, q, :], pooled_sb[h][:, k * 128:(k + 1) * 128],
                        ident)
                # each half is (k, h), (k+1, h): a strided [128, 2, 128] span
                for half in range(2):
                    k0, h0 = tr_jobs[4 * grp + 2 * half]
                    src = ps_tr[:, 2 * half:2 * half + 2, :]
                    dst = pooledT[:, k0:k0 + 2, h0 * 128:(h0 + 1) * 128]
                    if (grp + half) % 2 == 0:
                        nc.scalar.copy(dst, src)
                    else:
                        nc.vector.tensor_copy(dst, src)
            # small accumulators share one PSUM bank
            sm_ps = ps_sc.tile([128, 8], dt.float32, tag="sm", name="sm_ps")
            ps_sT = sm_ps[:, 0:2]
            # scores accumulate directly in slot-partition form: stationary =
            # hrelu pair slices (wide, satisfies fp8 dual-row Ldweights rules),
            # moving = w2 pairs -> out [slot, 1] per chunk-half. hrelu lives in
            # per-pair tiles so the score matmuls don't serialize against
            # later hrelu writes (dependencies are tile-granular).
            for j in range(HC // 2):
                ps_hh = ps_h.tile([128, 2, SLOTS], dt.float32, tag="psh",
                                  name=f"psh{j}")
                hrelu = ffn_pool.tile([128, 2, SLOTS], dt.float8e4,
                                      tag=f"hrelu{j}")
                for q in range(2):
                    hc = 2 * j + q
                    for k in range(0, KC + 2, 2):
                        nc.tensor.matmul(ps_hh[:, q, :],
                                         w1t[:, k:k + 2, hc * 128:(hc + 1) * 128],
                                         pooledT[:, k:k + 2, :],
                                         start=(k == 0), stop=(k == KC),
                                         perf_mode=mybir.MatmulPerfMode.DoubleRow)
                # W1,b1 host-scaled by 8 (b1 folded into w1t's ones-chunk row):
                # hrelu holds 8*h; 1/8 folded into the sigmoid scale below.
                # one wide eviction per pair, alternating engines
                if j % 2 == 0:
                    nc.scalar.activation(hrelu, ps_hh, AF.Relu)
                else:
                    nc.vector.tensor_scalar_max(hrelu, ps_hh, 0.0)
                w2p = w2t[:, 2 * j:2 * j + 2].rearrange("p (k o) -> p k o", o=1)
                for ch in range(2):
                    nc.tensor.matmul(
                        ps_sT[:, ch:ch + 1],
                        hrelu[:, :, ch * 128:(ch + 1) * 128],
                        w2p,
                        start=(j == 0), stop=(j == HC // 2 - 1),
                        perf_mode=mybir.MatmulPerfMode.DoubleRow)
            # W2 x16, W1/b1 x8 host scalings: sigmoid(psum/128 + b2)
            sT = ffn_pool.tile([128, 2], dt.float32, tag="sT")
            nc.scalar.activation(sT, ps_sT, AF.Sigmoid, bias=b2s,
                                 scale=0.0078125)
            ps_d = sm_ps[:NPAIR_SET, 2:4]
            for st in range(2):
                for ch in range(2):
                    nc.tensor.matmul(ps_d[:, st:st + 1],
                                     pairm[:, st, ch, :], sT[:, ch:ch + 1],
                                     start=(ch == 0), stop=(ch == 1))
            gamma_t = consts.tile([NPAIR_SET, 1], dt.float32)
            nc.vector.memset(gamma_t, GAMMA)
            relu_d = ffn_pool.tile([NPAIR_SET, 2], dt.float32, tag="relu_d")
            nc.scalar.activation(relu_d, ps_d, AF.Relu, bias=gamma_t)
            ones_t = consts.tile([NPAIR_SET, 1], dt.float32)
            nc.vector.memset(ones_t, 1.0)
            ps_l = sm_ps[0:1, 4:5]
            for st in range(2):
                nc.tensor.matmul(ps_l, relu_d[:, st:st + 1], ones_t,
                                 start=(st == 0), stop=(st == 1))
            loss_sb = ffn_pool.tile([1, 1], dt.float32, tag="loss")
            nc.vector.tensor_copy(loss_sb, ps_l)
            nc.sync.dma_start(out=loss_d[:, :], in_=loss_sb)

    _split_waits(nc)
    return nc


# ---------------------------------------------------------------- entry point

def kernel(triple_emb, W_sfa, W1, b1, W2, b2, tri2path_size):
    _patch_tile_drain()
    triple_emb = np.asarray(triple_emb, np.float32)
    sizes_flat = np.asarray(tri2path_size, np.int32).reshape(-1).astype(np.int64)
    offsets = np.concatenate([[0], np.cumsum(sizes_flat)[:-1]])

    core_rows, bins_all = _pack(sizes_flat)
    NB = max(len(b) for b in bins_all)
    MG = max(max(len(bn) for bn in bins) for bins in bins_all) + 1
    MG = ((MG + 7) // 8) * 8
    MG = max(MG, 40)

    triple_f8 = triple_emb.astype(bf16).astype(fp8e4)
    wsfa_t = np.ascontiguousarray(
        np.asarray(W_sfa, np.float32).T.reshape(KC, 128, D).transpose(1, 0, 2)
        .reshape(128, KC * D)).astype(fp8e4)
    w1_t = np.ascontiguousarray(
        (np.asarray(W1, np.float32) * 8.0).T.reshape(KC, 128, 4 * D)
        .transpose(1, 0, 2).reshape(128, KC * 4 * D)).astype(fp8e4)
    w2_t = np.ascontiguousarray(
        (np.asarray(W2, np.float32) * 16.0).reshape(HC, 128).T).astype(fp8e4)
    w1_b = (np.asarray(b1, np.float32) * 8.0).reshape(1, 4 * D).astype(fp8e4)
    b2_r = np.ascontiguousarray(
        np.broadcast_to(np.asarray(b2, np.float32).reshape(1, 1), (128, 1)))
    pair_m = np.zeros((128, 2, 2, NPAIR_SET), np.float32)
    for t in range(ROWS_PER_CORE * NEG):
        st, j = divmod(t, NPAIR_SET)
        b, k = divmod(t, NEG)
        slot_n = 16 * b + (k + 1)
        slot_p = 16 * b
        pair_m[slot_n % 128, st, slot_n // 128, j] += 1.0
        pair_m[slot_p % 128, st, slot_p // 128, j] -= 1.0

    in_maps = []
    for c in range(NCORES):
        # pad this core's bin list to NB bins (empty bins are all-masked)
        bins_c = bins_all[c] + [[]] * (NB - len(bins_all[c]))
        x, xt_flat, m, slot_of = _build_core_arrays(
            bins_c, triple_f8, offsets, NB, MG)
        in_maps.append({
            "x_bins": x, "xt_bins": xt_flat, "mask_f": m, "slot_of": slot_of,
            "w_sfa_t": wsfa_t, "w1_t": w1_t, "w2_t": w2_t,
            "w1_b": w1_b, "b2_r": b2_r, "pair_m": pair_m,
        })

    widths = _widths_of(NB)
    nunits_tot = sum((w // 2) + (w % 2) for w in widths)
    unit_halves = tuple(frozenset({0, 1}) for _ in range(nunits_tot))

    with _compile_lock:
        key = (widths, MG, unit_halves)
        nc = _compile_cache.get(key)
        if nc is None:
            nc = _build_program(widths, MG, unit_halves)
            _compile_cache[key] = nc

    res = run_bass_kernel_spmd(nc, in_maps, core_ids=list(range(NCORES)),
                               trace=bool(int(os.environ.get("KGE_TRACE", "0"))))
    total = np.float64(0.0)
    for r in res.results:
        total += np.float64(r["loss"][0, 0])
    kernel.last_results = res
    return np.asarray(np.float32(total))


# revision 71
# speedup vs baseline: 1.0012x; 1.0012x over previous
"""Trainium2 Bass kernel for nn_ContextKGEModel (self-attentive path pooling + FFN hinge loss).

Data-parallel over the 2048 ragged groups, 8 NeuronCores:
  - Host: assign 16 whole batch rows per core (load-balanced), pack each
    core's 256 groups into full 128-row bins with a greedy exact-fit packer
    (33 bins, zero padding for the canonical size distribution), and ship
    triple_emb in two fp8-e4m3 layouts (row-major bins with an appended ones
    column, and a d-major transposed copy in supertiles of up to 4 bins).
    The cross-group mask is encoded as a tiny per-bin +/-240 rank-G+1 factor
    pair so the mask is APPLIED BY THE TENSOR ENGINE as one extra accumulation
    matmul into the Gram PSUM (no mask DMA blocks, no vector mask-add).
    Weights are replicated and pre-transposed; W1/b1 host-scaled by 8 and W2
    by 16 so they stay in fp8 normal range (1/128 folds into the sigmoid
    scale). A +/-1 pair-selection matrix encodes the hinge pairs.
  - Device (per core): xwT = W_sfa^T @ X^T per supertile and per-bin Gram
    xw X^T run as fp8 DoubleRow matmuls; the group-masked column max is taken
    on the raw Gram (tanh is monotone so it commutes with max) with the mask
    already accumulated in PSUM; one fused tanh + one exp per supertile;
    attention weights are built ON THE GPSIMD ENGINE by an iota-vs-slot
    compare fused with the exp scale; unnormalized pooled vectors accumulate
    in PSUM across all bins (the ones column accumulates the softmax
    denominator); the FFN runs fp8 DoubleRow with the score reduction as
    DoubleRow pairs; the hinge loss is computed on-chip via the
    pair-selection matmul. PSUM->SBUF evictions alternate between the
    Activation and Vector engines so neither becomes the pipeline
    bottleneck. Host sums the 8 partial losses.
"""

import os
import threading
from contextlib import ExitStack

import numpy as np
import ml_dtypes

import concourse.bass as bass
import concourse.tile as tile
from concourse import mybir
from concourse.vector_clock import ScopedClock
from concourse.bass_utils import run_bass_kernel_spmd
from concourse.masks import make_identity

bf16 = ml_dtypes.bfloat16
fp8e4 = ml_dtypes.float8_e4m3

B, NEG, L, D = 128, 15, 32, 768
NPAIR_SET = 120                      # 240 hinge pairs split into 2 matmul sets
G = B * (NEG + 1)
GAMMA = 0.1
NCORES = 8
ROWS_PER_CORE = B // NCORES          # 16 batch rows / core
SLOTS = ROWS_PER_CORE * (NEG + 1)    # 256 group slots / core
BIN = 128
KC = D // 128                        # 6 contraction chunks
HC = (4 * D) // 128                  # 24 hidden chunks
DW = D + 8                           # x row + ones column + pad
NEG_MASK = -240.0

_compile_cache = {}
_compile_lock = threading.Lock()


def _patch_tile_drain():
    """This walrus build rejects >1 sem-wait on an instruction ("Too many sync
    wait commands"); split the TileContext tail-drain waits across SP nops."""
    if getattr(tile.TileContext, "_drain_patch_applied", False):
        return

    def _drain_and_barrier(self, tick_clock, wait_clock):
        probe = self.nc.sync.nop(nofuse=True, hint="drain_wait_split")
        wait_clock.add_sem_waits(probe.ins, ScopedClock({None: tick_clock.global_clock}))
        si = probe.ins.sync_info
        waits = list(si.on_wait) if si is not None and si.on_wait else []
        if len(waits) > 1:
            si.on_wait = waits[:1]
            for w in waits[1:]:
                extra = self.nc.sync.nop(nofuse=True, hint="drain_wait_split")
                esi = extra.ins.sync_info
                if esi is None:
                    extra.ins.sync_info = mybir.SyncInfo(on_wait=[w], on_update=[])
                else:
                    esi.on_wait = [w]
        self.nc.sync.drain()
        self.nc.all_engine_barrier()
        assert self.sems is not None
        popped = self.nc._tile_sem_poison_stack.pop()
        assert popped is self._sem_poison
        self.nc.clear_and_free_semaphores(list(self.sems.allocated().values()))
        self.nc.all_engine_barrier()

    tile.TileContext._drain_and_barrier = _drain_and_barrier
    tile.TileContext._drain_patch_applied = True


_MAX_WAITS = 1


def _split_waits(nc, maxw=_MAX_WAITS):
    """Hoist excess sync-waits onto NoOps inserted just before the
    instruction on the same engine (walrus build caps waits/instruction)."""
    n_split = 0
    for fn in nc.m.functions:
        for bb in fn.blocks:
            out = []
            for inst in bb.instructions:
                si = inst.sync_info
                waits = list(si.on_wait) if si is not None and si.on_wait else []
                if len(waits) > maxw:
                    keep = waits[:maxw]
                    rest = waits[maxw:]
                    for i in range(0, len(rest), maxw):
                        n_split += 1
                        nop = mybir.InstNoOp(
                            name=f"WSPLIT-{n_split}",
                            engine=inst.engine,
                            debug=inst.debug,
                            ins=[], outs=[],
                            sync_info=mybir.SyncInfo(
                                on_wait=rest[i:i + maxw], on_update=[]),
                        )
                        out.append(nop)
                    si.on_wait = keep
                out.append(inst)
            if n_split:
                bb.instructions[:] = out
    return n_split


# ---------------------------------------------------------------- host packing

def _pack(sizes_flat):
    """Balanced batch-row -> core assignment + greedy exact-fit bin packing
    (full 128-row bins; 33 bins/core for the canonical distribution)."""
    sizes = sizes_flat.reshape(B, NEG + 1)
    row_load = sizes.sum(1)
    order = np.argsort(-row_load, kind="stable")
    core_rows = [[] for _ in range(NCORES)]
    core_load = np.zeros(NCORES, np.int64)
    for b in order:
        cands = [c for c in range(NCORES) if len(core_rows[c]) < ROWS_PER_CORE]
        c = min(cands, key=lambda c: core_load[c])
        core_rows[c].append(int(b))
        core_load[c] += row_load[b]
    bins_all = []
    for c in range(NCORES):
        groups = []
        for lb, b in enumerate(core_rows[c]):
            for k in range(NEG + 1):
                g = b * (NEG + 1) + k
                groups.append((g, lb * (NEG + 1) + k, int(sizes_flat[g])))
        groups.sort(key=lambda t: -t[2])
        remaining = list(groups)
        bins = []
        while remaining:
            cap = BIN
            bn = []
            while cap > 0 and remaining:
                pick = None
                for idx, (g, slot, n) in enumerate(remaining):
                    if n == cap:
                        pick = idx
                        break
                if pick is None:
                    for idx, (g, slot, n) in enumerate(remaining):
                        if n <= cap:
                            pick = idx
                            break
                if pick is None:
                    break
                g, slot, n = remaining.pop(pick)
                bn.append((g, slot, n, BIN - cap))
                cap -= n
            bins.append(bn)
        bins_all.append(bins)
    return core_rows, bins_all


def _widths_of(nbins):
    """Supertile widths: as many 4-bin supertiles as possible + one tail."""
    w = [4] * (nbins // 4)
    if nbins % 4:
        w.append(nbins % 4)
    return tuple(w)


def _build_core_arrays(bins_c, triple_f8, offsets, NB, MG):
    """Per-core packed device inputs."""
    x = np.zeros((128, NB, DW), fp8e4)            # [row, bin, d] row-major + ones
    xt = np.zeros((128, NB, KC, BIN), np.float32)  # [dlane, bin, chunk, row] staging
    m = np.zeros((MG, NB, 2, BIN), fp8e4)          # mask factors M1 / M2
    slot_of = np.full((128, NB), -1.0, np.float32)
    for bi, bn in enumerate(bins_c):
        for qi, (g, slot, n, off) in enumerate(bn):
            rows = triple_f8[offsets[g]:offsets[g] + n]       # [n, D] fp8
            x[off:off + n, bi, :D] = rows
            x[off:off + n, bi, D] = 1.0
            xt_rows = rows.astype(np.float32).reshape(n, KC, 128)
            xt[:, bi, :, off:off + n] = xt_rows.transpose(2, 1, 0)
            slot_of[off:off + n, bi] = float(slot)
            # mask = 480*same - 480: fp8e4 caps at +-240, so the factor of 2
            # rides on the M2 side (240*2); -480 must beat the most negative
            # own-group Gram max (~ -6 sigma = -170) against cross-group cells
            m[qi, bi, 0, off:off + n] = 240.0
            m[qi, bi, 1, off:off + n] = 2.0
        m[MG - 1, bi, 0, :] = -240.0
        m[MG - 1, bi, 1, :] = 2.0
    xt8 = xt.astype(fp8e4)
    widths = _widths_of(NB)
    # xt flat layout: per supertile contiguous [dlane, chunk, w, row]
    blocks = []
    b0 = 0
    for w in widths:
        blk = xt8[:, b0:b0 + w].transpose(0, 2, 1, 3).reshape(128, KC * w * BIN)
        blocks.append(blk)
        b0 += w
    xt_flat = np.ascontiguousarray(np.concatenate(blocks, axis=1))
    return np.ascontiguousarray(x), xt_flat, np.ascontiguousarray(m), \
        np.ascontiguousarray(slot_of)


# ---------------------------------------------------------------- device program

def _build_program(widths, MG, unit_halves):
    NB = sum(widths)
    NST = len(widths)
    # pooled accumulation units: (st, bin_pair_or_single, local bins)
    units = []
    for s, w in enumerate(widths):
        for bp in range(w // 2):
            units.append((s, 2 * bp, 2))
        if w % 2:
            units.append((s, w - 1, 1))
    NU = len(units)
    # per-half accumulation chain membership for start/stop flags
    chain = {h: [u for u in range(NU) if h in unit_halves[u]] for h in (0, 1)}

    nc = bass.Bass()
    dt = mybir.dt
    AF = mybir.ActivationFunctionType

    x_d = nc.dram_tensor("x_bins", [128, NB, DW], dt.float8e4, kind="ExternalInput")
    xt_d = nc.dram_tensor("xt_bins", [128, KC * NB * BIN], dt.float8e4,
                          kind="ExternalInput")
    m_d = nc.dram_tensor("mask_f", [MG, NB, 2, BIN], dt.float8e4,
                         kind="ExternalInput")
    slot_d = nc.dram_tensor("slot_of", [128, NB], dt.float32, kind="ExternalInput")
    wsfa_d = nc.dram_tensor("w_sfa_t", [128, KC * D], dt.float8e4, kind="ExternalInput")
    w1t_d = nc.dram_tensor("w1_t", [128, KC * 4 * D], dt.float8e4, kind="ExternalInput")
    w2t_d = nc.dram_tensor("w2_t", [128, HC], dt.float8e4, kind="ExternalInput")
    w1b_d = nc.dram_tensor("w1_b", [1, 4 * D], dt.float8e4, kind="ExternalInput")
    b2_d = nc.dram_tensor("b2_r", [128, 1], dt.float32, kind="ExternalInput")
    pair_d = nc.dram_tensor("pair_m", [128, 2, 2, NPAIR_SET], dt.float32,
                            kind="ExternalInput")
    loss_d = nc.dram_tensor("loss", [1, 1], dt.float32, kind="ExternalOutput")

    st_off = []      # column offset of each supertile in xt_d / bin index base
    b0 = 0
    for w in widths:
        st_off.append(b0)
        b0 += w

    with tile.TileContext(nc) as tc, ExitStack() as ctx:
        consts = ctx.enter_context(tc.tile_pool(name="consts", bufs=1))
        xres = ctx.enter_context(tc.tile_pool(name="xres", bufs=1))
        attres = ctx.enter_context(tc.tile_pool(name="attres", bufs=1))
        xt_pool = ctx.enter_context(tc.tile_pool(name="xt", bufs=4))
        xwt_pool = ctx.enter_context(tc.tile_pool(name="xwt", bufs=4))
        small = ctx.enter_context(tc.tile_pool(name="small", bufs=12))
        cm_pool = ctx.enter_context(tc.tile_pool(name="cm", bufs=8))
        ffn_pool = ctx.enter_context(tc.tile_pool(name="ffn", bufs=1))

        # resident constants (wsfa + first supertile loads issued first so
        # compute starts as early as the serial DMA stream allows; wsfa comes
        # in 3 separately-tracked k-pair tiles so the first xw matmul only
        # waits on the first part)
        wsfa_k = [consts.tile([128, 2, D], dt.float8e4, tag=f"wsfa{i}",
                              name=f"wsfa{i}")
                  for i in range(KC // 2)]

        def load_wsfa(i):
            nc.sync.dma_start(
                out=wsfa_k[i],
                in_=wsfa_d[:, 2 * i * D:2 * (i + 1) * D].rearrange(
                    "p (k e) -> p k e", k=2))

        load_wsfa(0)

        x_tiles = [xres.tile([128, widths[s], DW], dt.float8e4, tag=f"x{s}",
                             name=f"x{s}") for s in range(NST)]
        # half-pure units only need a 128-wide attention window
        att_tiles = [attres.tile([128, nb, 128 * len(unit_halves[u])],
                                 dt.float8e4, tag=f"a{u}", name=f"a{u}")
                     for u, (_, _, nb) in enumerate(units)]

        xt_tiles = {}

        def load_xt(s):
            w = widths[s]
            xt_t = xt_pool.tile([128, KC, w * BIN], dt.float8e4, tag="xt",
                                name=f"xt{s}")
            off = KC * st_off[s] * BIN
            nc.sync.dma_start(
                out=xt_t,
                in_=xt_d[:, off:off + KC * w * BIN].rearrange(
                    "p (k c) -> p k c", k=KC))
            xt_tiles[s] = xt_t

        def load_x(s):
            # row-major x is only consumed by pooled (3 supertiles behind), so
            # its loads trail the xt stream instead of clogging the ramp
            w = widths[s]
            nc.sync.dma_start(out=x_tiles[s], in_=x_d[:, st_off[s]:st_off[s] + w, :])

        load_xt(0)
        load_wsfa(1)
        load_wsfa(2)
        load_xt(1)
        m_all = consts.tile([MG, NB, 2, BIN], dt.float8e4)
        nc.sync.dma_start(out=m_all, in_=m_d[:, :, :, :])
        slot_all = consts.tile([128, NB], dt.float32)
        nc.sync.dma_start(out=slot_all, in_=slot_d[:, :])
        ident = consts.tile([128, 128], dt.bfloat16)
        make_identity(nc, ident)
        # w1t carries 2 extra contraction chunks: chunk KC row 0 = b1*8 (pairs
        # with pooledT's ones row), chunk KC+1 = zero DoubleRow padding. The
        # zero regions are memset on the idle Pool engine during the DMA ramp;
        # the weight payloads stream in late (after the phase-A loads).
        w1t = consts.tile([128, KC + 2, 4 * D], dt.float8e4)
        nc.gpsimd.memset(w1t[:, KC:KC + 2, :], 0.0)
        iota_i = consts.tile([128, SLOTS], dt.int32)
        nc.gpsimd.iota(iota_i, pattern=[[1, SLOTS]], base=0, channel_multiplier=0)
        iota_f = consts.tile([128, SLOTS], dt.float32)
        nc.vector.tensor_copy(iota_f, iota_i)

        # ---- phase A: xwT per supertile; per-bin Gram+mask, fused tanh/exp,
        # gpsimd att build one supertile behind; pooled accumulation two
        # supertiles behind
        with (
            tc.tile_pool(name="ps_xw", bufs=3, space="PSUM") as ps_xw,
            tc.tile_pool(name="ps_gm", bufs=1, space="PSUM") as ps_gm,
            tc.tile_pool(name="ps_pool", bufs=1, space="PSUM") as ps_pooled,
        ):
            xwt_tiles = {}
            evict_flip = [0]

            def emit_xw(s):
                w = widths[s]
                xt_t = xt_tiles[s]
                xwt_t = xwt_pool.tile([128, KC, w * BIN], dt.float8e4,
                                      tag="xwt", name=f"xwt{s}")
                for e in range(KC):
                    ps = ps_xw.tile([128, 4 * BIN], dt.float32, tag="psxw",
                                    name=f"psxw{s}_{e}")
                    for k in range(0, KC, 2):
                        nc.tensor.matmul(
                            ps[:, :w * BIN],
                            wsfa_k[k // 2][:, :, e * 128:(e + 1) * 128],
                            xt_t[:, k:k + 2, :],
                            start=(k == 0), stop=(k == KC - 2),
                            perf_mode=mybir.MatmulPerfMode.DoubleRow)
                    if evict_flip[0] % 2 == 0:
                        nc.scalar.copy(xwt_t[:, e, :], ps[:, :w * BIN])
                    else:
                        nc.vector.tensor_copy(xwt_t[:, e, :], ps[:, :w * BIN])
                    evict_flip[0] += 1
                xwt_tiles[s] = xwt_t

            unit_base = {}
            ub = 0
            for s, w in enumerate(widths):
                unit_base[s] = ub
                ub += (w // 2) + (w % 2)

            def emit_bins(s):
                w = widths[s]
                xt_t, xwt_t = xt_tiles[s], xwt_tiles[s]
                ps_g = ps_gm.tile([128, 4, BIN], dt.float32, tag="psgm",
                                  name=f"psgm{s}")
                for lb in range(w):
                    bi = st_off[s] + lb
                    sl = slice(lb * BIN, (lb + 1) * BIN)
                    for e in range(0, KC, 2):
                        nc.tensor.matmul(ps_g[:, lb, :], xwt_t[:, e:e + 2, sl],
                                         xt_t[:, e:e + 2, sl],
                                         start=(e == 0), stop=False,
                                         perf_mode=mybir.MatmulPerfMode.DoubleRow)
                    # cross-group mask as one accumulation matmul:
                    # M1^T M2 = 240*same - 240
                    nc.tensor.matmul(ps_g[:, lb, :], m_all[:, bi, 0, :],
                                     m_all[:, bi, 1, :],
                                     start=False, stop=True)
                # masked max of raw Gram; tanh applied after the max
                # (tanh is monotone, so max commutes with it); latency of this
                # fused chain is hidden by the 3-supertile pooled distance
                cm = cm_pool.tile([128, 4], dt.float32, tag="cm", name=f"cm{s}")
                nc.vector.tensor_reduce(
                    out=cm[:, :w], in_=ps_g[:, :w, :],
                    op=mybir.AluOpType.max, axis=mybir.AxisListType.X)
                th = cm_pool.tile([128, 4], dt.float32, tag="th", name=f"th{s}")
                nc.scalar.activation(th[:, :w], cm[:, :w], AF.Tanh)
                ex = cm_pool.tile([128, 4], dt.float32, tag="ex", name=f"ex{s}")
                nc.scalar.activation(ex[:, :w], th[:, :w], AF.Exp)
                nunits = (w // 2) + (w % 2)
                # last supertiles' att on DVE: Pool's serial backlog would
                # otherwise gate the final pooled accumulations
                att_eng = nc.vector if s >= NST - 2 else nc.gpsimd
                for ui in range(nunits):
                    lb0 = 2 * ui
                    nb = 2 if lb0 + 1 < w else 1
                    u = unit_base[s] + ui
                    att_t = att_tiles[u]
                    halves = sorted(unit_halves[u])
                    io_sl = (slice(halves[0] * 128, (halves[0] + 1) * 128)
                             if len(halves) == 1 else slice(0, SLOTS))
                    for j in range(nb):
                        bi = st_off[s] + lb0 + j
                        att_eng.tensor_scalar(
                            out=att_t[:, j, :], in0=iota_f[:, io_sl],
                            scalar1=slot_all[:, bi:bi + 1],
                            scalar2=ex[:, lb0 + j:lb0 + j + 1],
                            op0=mybir.AluOpType.is_equal,
                            op1=mybir.AluOpType.mult)

            ps_p = [ps_pooled.tile([128, DW], dt.float32, tag=f"psp{h}",
                                   name=f"psp{h}") for h in range(2)]

            def emit_pooled(s):
                w = widths[s]
                xv = x_tiles[s]
                nunits = (w // 2) + (w % 2)
                for ui in range(nunits):
                    u = unit_base[s] + ui
                    lb0 = 2 * ui
                    nb = units[u][2]
                    att_t = att_tiles[u]
                    halves = sorted(unit_halves[u])
                    kw = ({"perf_mode": mybir.MatmulPerfMode.DoubleRow}
                          if nb == 2 else {})
                    for h in halves:
                        hsl = (slice(0, 128) if len(halves) == 1
                               else slice(h * 128, (h + 1) * 128))
                        # keep each matmul output inside one PSUM bank
                        for n0, nlen in ((0, 512), (512, DW - 512)):
                            nc.tensor.matmul(
                                ps_p[h][:, n0:n0 + nlen],
                                att_t[:, :, hsl],
                                xv[:, lb0:lb0 + nb, n0:n0 + nlen],
                                start=(u == chain[h][0]),
                                stop=(u == chain[h][-1]), **kw)

            for s in range(NST):
                emit_xw(s)
                if s + 2 < NST:
                    load_xt(s + 2)
                load_x(s)
                if s >= 1:
                    emit_bins(s - 1)
                if s >= 3:
                    emit_pooled(s - 3)
            emit_bins(NST - 1)
            emit_pooled(NST - 3)
            emit_pooled(NST - 2)
            emit_pooled(NST - 1)

            # FFN weights loaded late so they don't delay the phase-A DMA stream
            nc.sync.dma_start(out=w1t[:, :KC, :],
                              in_=w1t_d[:, :].rearrange("p (k h) -> p k h", k=KC))
            nc.sync.dma_start(out=w1t[0:1, KC, :], in_=w1b_d[:, :])
            w2t = consts.tile([128, HC], dt.float8e4)
            nc.sync.dma_start(out=w2t, in_=w2t_d[:, :])
            b2s = consts.tile([128, 1], dt.float32)
            nc.sync.dma_start(out=b2s, in_=b2_d[:, :])
            pairm = consts.tile([128, 2, 2, NPAIR_SET], dt.float32)
            nc.sync.dma_start(out=pairm, in_=pair_d[:, :, :, :])

            # ---- phase B1: normalize pooled by the accumulated denominator
            # (separate tiles per slot-half so each half's transposes only
            # wait on its own normalize)
            pooled_sb = [ffn_pool.tile([128, D], dt.bfloat16, tag=f"pooled{h}",
                                       name=f"pooled{h}") for h in range(2)]
            for h in range(2):
                rz = small.tile([128, 1], dt.float32, tag="rz", name=f"rz{h}")
                nc.vector.reciprocal(rz, ps_p[h][:, D:D + 1])
                if h == 0:
                    nc.scalar.activation(pooled_sb[h], ps_p[h][:, :D],
                                         AF.Copy, scale=rz)
                else:
                    nc.vector.tensor_scalar_mul(pooled_sb[h],
                                                ps_p[h][:, :D], rz)

        # ---- phase B2: transpose pooled, FFN, hinge loss
        with (
            tc.tile_pool(name="ps_t", bufs=3, space="PSUM") as ps_t,
            tc.tile_pool(name="ps_h", bufs=3, space="PSUM") as ps_h,
            tc.tile_pool(name="ps_sc", bufs=1, space="PSUM") as ps_sc,
        ):
            # transpose pooled in batches of 4: one [128, 4*128] PSUM tile per
            # eviction so the PSUM->SBUF copies are few and wide. pooledT has
            # 2 extra contraction chunks: chunk KC carries a ones row at
            # partition 0 (pairs with the b1 row folded into w1t on host) and
            # chunk KC+1 is zero padding for the DoubleRow pair.
            pooledT = ffn_pool.tile([128, KC + 2, SLOTS], dt.float8e4,
                                    tag="pooledT")
            nc.gpsimd.memset(pooledT[:, KC:KC + 2, :], 0.0)
            nc.gpsimd.memset(pooledT[0:1, KC, :], 1.0)
            # h-major so the first transposes only wait on h0's normalize
            tr_jobs = [(k, h) for h in range(2) for k in range(KC)]
            for grp in range(3):
                ps_tr = ps_t.tile([128, 4, 128], dt.bfloat16, tag="pstr",
                                  name=f"pstr{grp}")
                for q in range(4):
                    k, h = tr_jobs[4 * grp + q]
                    nc.tensor.transpose(
                        ps_tr[:, q, :], pooled_sb[h][:, k * 128:(k + 1) * 128],
                        ident)
                # each half is (k, h), (k+1, h): a strided [128, 2, 128] span
                for half in range(2):
                    k0, h0 = tr_jobs[4 * grp + 2 * half]
                    src = ps_tr[:, 2 * half:2 * half + 2, :]
                    dst = pooledT[:, k0:k0 + 2, h0 * 128:(h0 + 1) * 128]
                    if (grp + half) % 2 == 0:
                        nc.scalar.copy(dst, src)
                    else:
                        nc.vector.tensor_copy(dst, src)
            # small accumulators share one PSUM bank
            sm_ps = ps_sc.tile([128, 8], dt.float32, tag="sm", name="sm_ps")
            ps_sT = sm_ps[:, 0:2]
            # scores accumulate directly in slot-partition form: stationary =
            # hrelu pair slices (wide, satisfies fp8 dual-row Ldweights rules),
            # moving = w2 pairs -> out [slot, 1] per chunk-half. hrelu lives in
            # per-pair tiles so the score matmuls don't serialize against
            # later hrelu writes (dependencies are tile-granular).
            for j in range(HC // 2):
                ps_hh = ps_h.tile([128, 2, SLOTS], dt.float32, tag="psh",
                                  name=f"psh{j}")
                hrelu = ffn_pool.tile([128, 2, SLOTS], dt.float8e4,
                                      tag=f"hrelu{j}")
                for q in range(2):
                    hc = 2 * j + q
                    for k in range(0, KC + 2, 2):
                        nc.tensor.matmul(ps_hh[:, q, :],
                                         w1t[:, k:k + 2, hc * 128:(hc + 1) * 128],
                                         pooledT[:, k:k + 2, :],
                                         start=(k == 0), stop=(k == KC),
                                         perf_mode=mybir.MatmulPerfMode.DoubleRow)
                # W1,b1 host-scaled by 8 (b1 folded into w1t's ones-chunk row):
                # hrelu holds 8*h; 1/8 folded into the sigmoid scale below.
                # one wide eviction per pair, alternating engines
                if j % 2 == 0:
                    nc.scalar.activation(hrelu, ps_hh, AF.Relu)
                else:
                    nc.vector.tensor_scalar_max(hrelu, ps_hh, 0.0)
                w2p = w2t[:, 2 * j:2 * j + 2].rearrange("p (k o) -> p k o", o=1)
                for ch in range(2):
                    nc.tensor.matmul(
                        ps_sT[:, ch:ch + 1],
                        hrelu[:, :, ch * 128:(ch + 1) * 128],
                        w2p,
                        start=(j == 0), stop=(j == HC // 2 - 1),
                        perf_mode=mybir.MatmulPerfMode.DoubleRow)
            # W2 x16, W1/b1 x8 host scalings: sigmoid(psum/128 + b2)
            sT = ffn_pool.tile([128, 2], dt.float32, tag="sT")
            nc.scalar.activation(sT, ps_sT, AF.Sigmoid, bias=b2s,
                                 scale=0.0078125)
            ps_d = sm_ps[:NPAIR_SET, 2:4]
            for st in range(2):
                for ch in range(2):
                    nc.tensor.matmul(ps_d[:, st:st + 1],
                                     pairm[:, st, ch, :], sT[:, ch:ch + 1],
                                     start=(ch == 0), stop=(ch == 1))
            relu_d = ffn_pool.tile([NPAIR_SET, 2], dt.float32, tag="relu_d")
            nc.vector.tensor_scalar(out=relu_d, in0=ps_d, scalar1=GAMMA,
                                    scalar2=0.0, op0=mybir.AluOpType.add,
                                    op1=mybir.AluOpType.max)
            ones_t = consts.tile([NPAIR_SET, 1], dt.float32)
            nc.vector.memset(ones_t, 1.0)
            ps_l = sm_ps[0:1, 4:5]
            for st in range(2):
                nc.tensor.matmul(ps_l, relu_d[:, st:st + 1], ones_t,
                                 start=(st == 0), stop=(st == 1))
            loss_sb = ffn_pool.tile([1, 1], dt.float32, tag="loss")
            nc.scalar.activation(loss_sb, ps_l, AF.Copy)
            nc.sync.dma_start(out=loss_d[:, :], in_=loss_sb)

    _split_waits(nc)
    return nc


# ---------------------------------------------------------------- entry point

def kernel(triple_emb, W_sfa, W1, b1, W2, b2, tri2path_size):
    _patch_tile_drain()
    triple_emb = np.asarray(triple_emb, np.float32)
    sizes_flat = np.asarray(tri2path_size, np.int32).reshape(-1).astype(np.int64)
    offsets = np.concatenate([[0], np.cumsum(sizes_flat)[:-1]])

    core_rows, bins_all = _pack(sizes_flat)
    NB = max(len(b) for b in bins_all)
    MG = max(max(len(bn) for bn in bins) for bins in bins_all) + 1
    MG = ((MG + 7) // 8) * 8
    MG = max(MG, 40)

    triple_f8 = triple_emb.astype(bf16).astype(fp8e4)
    wsfa_t = np.ascontiguousarray(
        np.asarray(W_sfa, np.float32).T.reshape(KC, 128, D).transpose(1, 0, 2)
        .reshape(128, KC * D)).astype(fp8e4)
    w1_t = np.ascontiguousarray(
        (np.asarray(W1, np.float32) * 8.0).T.reshape(KC, 128, 4 * D)
        .transpose(1, 0, 2).reshape(128, KC * 4 * D)).astype(fp8e4)
    w2_t = np.ascontiguousarray(
        (np.asarray(W2, np.float32) * 16.0).reshape(HC, 128).T).astype(fp8e4)
    w1_b = (np.asarray(b1, np.float32) * 8.0).reshape(1, 4 * D).astype(fp8e4)
    b2_r = np.ascontiguousarray(
        np.broadcast_to(np.asarray(b2, np.float32).reshape(1, 1), (128, 1)))
    pair_m = np.zeros((128, 2, 2, NPAIR_SET), np.float32)
    for t in range(ROWS_PER_CORE * NEG):
        st, j = divmod(t, NPAIR_SET)
        b, k = divmod(t, NEG)
        slot_n = 16 * b + (k + 1)
        slot_p = 16 * b
        pair_m[slot_n % 128, st, slot_n // 128, j] += 1.0
        pair_m[slot_p % 128, st, slot_p // 128, j] -= 1.0

    in_maps = []
    for c in range(NCORES):
        # pad this core's bin list to NB bins (empty bins are all-masked)
        bins_c = bins_all[c] + [[]] * (NB - len(bins_all[c]))
        x, xt_flat, m, slot_of = _build_core_arrays(
            bins_c, triple_f8, offsets, NB, MG)
        in_maps.append({
            "x_bins": x, "xt_bins": xt_flat, "mask_f": m, "slot_of": slot_of,
            "w_sfa_t": wsfa_t, "w1_t": w1_t, "w2_t": w2_t,
            "w1_b": w1_b, "b2_r": b2_r, "pair_m": pair_m,
        })

    widths = _widths_of(NB)
    nunits_tot = sum((w // 2) + (w % 2) for w in widths)
    unit_halves = tuple(frozenset({0, 1}) for _ in range(nunits_tot))

    with _compile_lock:
        key = (widths, MG, unit_halves)
        nc = _compile_cache.get(key)
        if nc is None:
            nc = _build_program(widths, MG, unit_halves)
            _compile_cache[key] = nc

    res = run_bass_kernel_spmd(nc, in_maps, core_ids=list(range(NCORES)),
                               trace=bool(int(os.environ.get("KGE_TRACE", "0"))))
    total = np.float64(0.0)
    for r in res.results:
        total += np.float64(r["loss"][0, 0])
    kernel.last_results = res
    return np.asarray(np.float32(total))


# revision 73
# speedup vs baseline: 1.0210x; 1.0198x over previous
"""Trainium2 Bass kernel for nn_ContextKGEModel (self-attentive path pooling + FFN hinge loss).

Data-parallel over the 2048 ragged groups, 8 NeuronCores:
  - Host: assign 16 whole batch rows per core (load-balanced), pack each
    core's 256 groups into full 128-row bins with a greedy exact-fit packer
    (33 bins, zero padding for the canonical size distribution), and ship
    triple_emb in two fp8-e4m3 layouts (row-major bins with an appended ones
    column, and a d-major transposed copy in supertiles of up to 4 bins).
    The cross-group mask is encoded as a tiny per-bin +/-240 rank-G+1 factor
    pair so the mask is APPLIED BY THE TENSOR ENGINE as one extra accumulation
    matmul into the Gram PSUM (no mask DMA blocks, no vector mask-add).
    Weights are replicated and pre-transposed; W1/b1 host-scaled by 8 and W2
    by 16 so they stay in fp8 normal range (1/128 folds into the sigmoid
    scale). A +/-1 pair-selection matrix encodes the hinge pairs.
  - Device (per core): xwT = W_sfa^T @ X^T per supertile and per-bin Gram
    xw X^T run as fp8 DoubleRow matmuls; the group-masked column max is taken
    on the raw Gram (tanh is monotone so it commutes with max) with the mask
    already accumulated in PSUM; one fused tanh + one exp per supertile;
    attention weights are built ON THE GPSIMD ENGINE by an iota-vs-slot
    compare fused with the exp scale; unnormalized pooled vectors accumulate
    in PSUM across all bins (the ones column accumulates the softmax
    denominator); the FFN runs fp8 DoubleRow with the score reduction as
    DoubleRow pairs; the hinge loss is computed on-chip via the
    pair-selection matmul. PSUM->SBUF evictions alternate between the
    Activation and Vector engines so neither becomes the pipeline
    bottleneck. Host sums the 8 partial losses.
"""

import os
import threading
from contextlib import ExitStack

import numpy as np
import ml_dtypes

import concourse.bass as bass
import concourse.tile as tile
from concourse import mybir
from concourse.vector_clock import ScopedClock
from concourse.bass_utils import run_bass_kernel_spmd
from concourse.masks import make_identity

bf16 = ml_dtypes.bfloat16
fp8e4 = ml_dtypes.float8_e4m3

B, NEG, L, D = 128, 15, 32, 768
NPAIR_SET = 120                      # 240 hinge pairs split into 2 matmul sets
G = B * (NEG + 1)
GAMMA = 0.1
NCORES = 8
ROWS_PER_CORE = B // NCORES          # 16 batch rows / core
SLOTS = ROWS_PER_CORE * (NEG + 1)    # 256 group slots / core
BIN = 128
KC = D // 128                        # 6 contraction chunks
HC = (4 * D) // 128                  # 24 hidden chunks
DW = D + 8                           # x row + ones column + pad
NEG_MASK = -240.0

_compile_cache = {}
_compile_lock = threading.Lock()


def _patch_tile_drain():
    """This walrus build rejects >1 sem-wait on an instruction ("Too many sync
    wait commands"); split the TileContext tail-drain waits across SP nops."""
    if getattr(tile.TileContext, "_drain_patch_applied", False):
        return

    def _drain_and_barrier(self, tick_clock, wait_clock):
        probe = self.nc.sync.nop(nofuse=True, hint="drain_wait_split")
        wait_clock.add_sem_waits(probe.ins, ScopedClock({None: tick_clock.global_clock}))
        si = probe.ins.sync_info
        waits = list(si.on_wait) if si is not None and si.on_wait else []
        if len(waits) > 1:
            si.on_wait = waits[:1]
            for w in waits[1:]:
                extra = self.nc.sync.nop(nofuse=True, hint="drain_wait_split")
                esi = extra.ins.sync_info
                if esi is None:
                    extra.ins.sync_info = mybir.SyncInfo(on_wait=[w], on_update=[])
                else:
                    esi.on_wait = [w]
        self.nc.sync.drain()
        self.nc.all_engine_barrier()
        assert self.sems is not None
        popped = self.nc._tile_sem_poison_stack.pop()
        assert popped is self._sem_poison
        self.nc.clear_and_free_semaphores(list(self.sems.allocated().values()))
        self.nc.all_engine_barrier()

    tile.TileContext._drain_and_barrier = _drain_and_barrier
    tile.TileContext._drain_patch_applied = True


_MAX_WAITS = 1


def _split_waits(nc, maxw=_MAX_WAITS):
    """Hoist excess sync-waits onto NoOps inserted just before the
    instruction on the same engine (walrus build caps waits/instruction)."""
    n_split = 0
    for fn in nc.m.functions:
        for bb in fn.blocks:
            out = []
            for inst in bb.instructions:
                si = inst.sync_info
                waits = list(si.on_wait) if si is not None and si.on_wait else []
                if len(waits) > maxw:
                    keep = waits[:maxw]
                    rest = waits[maxw:]
                    for i in range(0, len(rest), maxw):
                        n_split += 1
                        nop = mybir.InstNoOp(
                            name=f"WSPLIT-{n_split}",
                            engine=inst.engine,
                            debug=inst.debug,
                            ins=[], outs=[],
                            sync_info=mybir.SyncInfo(
                                on_wait=rest[i:i + maxw], on_update=[]),
                        )
                        out.append(nop)
                    si.on_wait = keep
                out.append(inst)
            if n_split:
                bb.instructions[:] = out
    return n_split


# ---------------------------------------------------------------- host packing

def _pack(sizes_flat):
    """Balanced batch-row -> core assignment + greedy exact-fit bin packing
    (full 128-row bins; 33 bins/core for the canonical distribution)."""
    sizes = sizes_flat.reshape(B, NEG + 1)
    row_load = sizes.sum(1)
    order = np.argsort(-row_load, kind="stable")
    core_rows = [[] for _ in range(NCORES)]
    core_load = np.zeros(NCORES, np.int64)
    for b in order:
        cands = [c for c in range(NCORES) if len(core_rows[c]) < ROWS_PER_CORE]
        c = min(cands, key=lambda c: core_load[c])
        core_rows[c].append(int(b))
        core_load[c] += row_load[b]
    bins_all = []
    for c in range(NCORES):
        groups = []
        for lb, b in enumerate(core_rows[c]):
            for k in range(NEG + 1):
                g = b * (NEG + 1) + k
                groups.append((g, lb * (NEG + 1) + k, int(sizes_flat[g])))
        groups.sort(key=lambda t: -t[2])
        remaining = list(groups)
        bins = []
        while remaining:
            cap = BIN
            bn = []
            while cap > 0 and remaining:
                pick = None
                for idx, (g, slot, n) in enumerate(remaining):
                    if n == cap:
                        pick = idx
                        break
                if pick is None:
                    for idx, (g, slot, n) in enumerate(remaining):
                        if n <= cap:
                            pick = idx
                            break
                if pick is None:
                    break
                g, slot, n = remaining.pop(pick)
                bn.append((g, slot, n, BIN - cap))
                cap -= n
            bins.append(bn)
        bins_all.append(bins)
    return core_rows, bins_all


def _widths_of(nbins):
    """Supertile widths: as many 4-bin supertiles as possible + one tail."""
    w = [4] * (nbins // 4)
    if nbins % 4:
        w.append(nbins % 4)
    return tuple(w)


def _build_core_arrays(bins_c, triple_f8, offsets, NB, MG):
    """Per-core packed device inputs."""
    x = np.zeros((128, NB, DW), fp8e4)            # [row, bin, d] row-major + ones
    xt = np.zeros((128, NB, KC, BIN), np.float32)  # [dlane, bin, chunk, row] staging
    m = np.zeros((MG, NB, 2, BIN), fp8e4)          # mask factors M1 / M2
    slot_of = np.full((128, NB), -1.0, np.float32)
    for bi, bn in enumerate(bins_c):
        for qi, (g, slot, n, off) in enumerate(bn):
            rows = triple_f8[offsets[g]:offsets[g] + n]       # [n, D] fp8
            x[off:off + n, bi, :D] = rows
            x[off:off + n, bi, D] = 1.0
            xt_rows = rows.astype(np.float32).reshape(n, KC, 128)
            xt[:, bi, :, off:off + n] = xt_rows.transpose(2, 1, 0)
            slot_of[off:off + n, bi] = float(slot)
            # mask = 480*same - 480: fp8e4 caps at +-240, so the factor of 2
            # rides on the M2 side (240*2); -480 must beat the most negative
            # own-group Gram max (~ -6 sigma = -170) against cross-group cells
            m[qi, bi, 0, off:off + n] = 240.0
            m[qi, bi, 1, off:off + n] = 2.0
        m[MG - 1, bi, 0, :] = -240.0
        m[MG - 1, bi, 1, :] = 2.0
    xt8 = xt.astype(fp8e4)
    widths = _widths_of(NB)
    # xt flat layout: per supertile contiguous [dlane, chunk, w, row]
    blocks = []
    b0 = 0
    for w in widths:
        blk = xt8[:, b0:b0 + w].transpose(0, 2, 1, 3).reshape(128, KC * w * BIN)
        blocks.append(blk)
        b0 += w
    xt_flat = np.ascontiguousarray(np.concatenate(blocks, axis=1))
    return np.ascontiguousarray(x), xt_flat, np.ascontiguousarray(m), \
        np.ascontiguousarray(slot_of)


# ---------------------------------------------------------------- device program

def _build_program(widths, MG, unit_halves):
    NB = sum(widths)
    NST = len(widths)
    # pooled accumulation units: (st, bin_pair_or_single, local bins)
    units = []
    for s, w in enumerate(widths):
        for bp in range(w // 2):
            units.append((s, 2 * bp, 2))
        if w % 2:
            units.append((s, w - 1, 1))
    NU = len(units)
    # per-half accumulation chain membership for start/stop flags
    chain = {h: [u for u in range(NU) if h in unit_halves[u]] for h in (0, 1)}

    nc = bass.Bass()
    dt = mybir.dt
    AF = mybir.ActivationFunctionType

    x_d = nc.dram_tensor("x_bins", [128, NB, DW], dt.float8e4, kind="ExternalInput")
    xt_d = nc.dram_tensor("xt_bins", [128, KC * NB * BIN], dt.float8e4,
                          kind="ExternalInput")
    m_d = nc.dram_tensor("mask_f", [MG, NB, 2, BIN], dt.float8e4,
                         kind="ExternalInput")
    slot_d = nc.dram_tensor("slot_of", [128, NB], dt.float32, kind="ExternalInput")
    wsfa_d = nc.dram_tensor("w_sfa_t", [128, KC * D], dt.float8e4, kind="ExternalInput")
    w1t_d = nc.dram_tensor("w1_t", [128, KC * 4 * D], dt.float8e4, kind="ExternalInput")
    w2t_d = nc.dram_tensor("w2_t", [128, HC], dt.float8e4, kind="ExternalInput")
    w1b_d = nc.dram_tensor("w1_b", [1, 4 * D], dt.float8e4, kind="ExternalInput")
    loss_d = nc.dram_tensor("scores", [128, 2], dt.float32, kind="ExternalOutput")

    st_off = []      # column offset of each supertile in xt_d / bin index base
    b0 = 0
    for w in widths:
        st_off.append(b0)
        b0 += w

    with tile.TileContext(nc) as tc, ExitStack() as ctx:
        consts = ctx.enter_context(tc.tile_pool(name="consts", bufs=1))
        xres = ctx.enter_context(tc.tile_pool(name="xres", bufs=1))
        attres = ctx.enter_context(tc.tile_pool(name="attres", bufs=1))
        xt_pool = ctx.enter_context(tc.tile_pool(name="xt", bufs=4))
        xwt_pool = ctx.enter_context(tc.tile_pool(name="xwt", bufs=4))
        small = ctx.enter_context(tc.tile_pool(name="small", bufs=12))
        cm_pool = ctx.enter_context(tc.tile_pool(name="cm", bufs=8))
        ffn_pool = ctx.enter_context(tc.tile_pool(name="ffn", bufs=1))

        # resident constants (wsfa + first supertile loads issued first so
        # compute starts as early as the serial DMA stream allows; wsfa comes
        # in 3 separately-tracked k-pair tiles so the first xw matmul only
        # waits on the first part)
        wsfa_k = [consts.tile([128, 2, D], dt.float8e4, tag=f"wsfa{i}",
                              name=f"wsfa{i}")
                  for i in range(KC // 2)]

        def load_wsfa(i):
            nc.sync.dma_start(
                out=wsfa_k[i],
                in_=wsfa_d[:, 2 * i * D:2 * (i + 1) * D].rearrange(
                    "p (k e) -> p k e", k=2))

        load_wsfa(0)

        x_tiles = [xres.tile([128, widths[s], DW], dt.float8e4, tag=f"x{s}",
                             name=f"x{s}") for s in range(NST)]
        # half-pure units only need a 128-wide attention window
        att_tiles = [attres.tile([128, nb, 128 * len(unit_halves[u])],
                                 dt.float8e4, tag=f"a{u}", name=f"a{u}")
                     for u, (_, _, nb) in enumerate(units)]

        xt_tiles = {}

        def load_xt(s):
            w = widths[s]
            xt_t = xt_pool.tile([128, KC, w * BIN], dt.float8e4, tag="xt",
                                name=f"xt{s}")
            off = KC * st_off[s] * BIN
            nc.sync.dma_start(
                out=xt_t,
                in_=xt_d[:, off:off + KC * w * BIN].rearrange(
                    "p (k c) -> p k c", k=KC))
            xt_tiles[s] = xt_t

        def load_x(s):
            # row-major x is only consumed by pooled (3 supertiles behind), so
            # its loads trail the xt stream instead of clogging the ramp
            w = widths[s]
            nc.sync.dma_start(out=x_tiles[s], in_=x_d[:, st_off[s]:st_off[s] + w, :])

        load_xt(0)
        load_wsfa(1)
        load_wsfa(2)
        load_xt(1)
        m_all = consts.tile([MG, NB, 2, BIN], dt.float8e4)
        nc.sync.dma_start(out=m_all, in_=m_d[:, :, :, :])
        slot_all = consts.tile([128, NB], dt.float32)
        nc.sync.dma_start(out=slot_all, in_=slot_d[:, :])
        ident = consts.tile([128, 128], dt.bfloat16)
        make_identity(nc, ident)
        # w1t carries 2 extra contraction chunks: chunk KC row 0 = b1*8 (pairs
        # with pooledT's ones row), chunk KC+1 = zero DoubleRow padding. The
        # zero regions are memset on the idle Pool engine during the DMA ramp;
        # the weight payloads stream in late (after the phase-A loads).
        w1t = consts.tile([128, KC + 2, 4 * D], dt.float8e4)
        nc.gpsimd.memset(w1t[:, KC:KC + 2, :], 0.0)
        iota_i = consts.tile([128, SLOTS], dt.int32)
        nc.gpsimd.iota(iota_i, pattern=[[1, SLOTS]], base=0, channel_multiplier=0)
        iota_f = consts.tile([128, SLOTS], dt.float32)
        nc.vector.tensor_copy(iota_f, iota_i)

        # ---- phase A: xwT per supertile; per-bin Gram+mask, fused tanh/exp,
        # gpsimd att build one supertile behind; pooled accumulation two
        # supertiles behind
        with (
            tc.tile_pool(name="ps_xw", bufs=3, space="PSUM") as ps_xw,
            tc.tile_pool(name="ps_gm", bufs=1, space="PSUM") as ps_gm,
            tc.tile_pool(name="ps_pool", bufs=1, space="PSUM") as ps_pooled,
        ):
            xwt_tiles = {}
            evict_flip = [0]

            def emit_xw(s):
                w = widths[s]
                xt_t = xt_tiles[s]
                xwt_t = xwt_pool.tile([128, KC, w * BIN], dt.float8e4,
                                      tag="xwt", name=f"xwt{s}")
                for e in range(KC):
                    ps = ps_xw.tile([128, 4 * BIN], dt.float32, tag="psxw",
                                    name=f"psxw{s}_{e}")
                    for k in range(0, KC, 2):
                        nc.tensor.matmul(
                            ps[:, :w * BIN],
                            wsfa_k[k // 2][:, :, e * 128:(e + 1) * 128],
                            xt_t[:, k:k + 2, :],
                            start=(k == 0), stop=(k == KC - 2),
                            perf_mode=mybir.MatmulPerfMode.DoubleRow)
                    if evict_flip[0] % 2 == 0:
                        nc.scalar.copy(xwt_t[:, e, :], ps[:, :w * BIN])
                    else:
                        nc.vector.tensor_copy(xwt_t[:, e, :], ps[:, :w * BIN])
                    evict_flip[0] += 1
                xwt_tiles[s] = xwt_t

            unit_base = {}
            ub = 0
            for s, w in enumerate(widths):
                unit_base[s] = ub
                ub += (w // 2) + (w % 2)

            def emit_bins(s):
                w = widths[s]
                xt_t, xwt_t = xt_tiles[s], xwt_tiles[s]
                ps_g = ps_gm.tile([128, 4, BIN], dt.float32, tag="psgm",
                                  name=f"psgm{s}")
                for lb in range(w):
                    bi = st_off[s] + lb
                    sl = slice(lb * BIN, (lb + 1) * BIN)
                    for e in range(0, KC, 2):
                        nc.tensor.matmul(ps_g[:, lb, :], xwt_t[:, e:e + 2, sl],
                                         xt_t[:, e:e + 2, sl],
                                         start=(e == 0), stop=False,
                                         perf_mode=mybir.MatmulPerfMode.DoubleRow)
                    # cross-group mask as one accumulation matmul:
                    # M1^T M2 = 240*same - 240
                    nc.tensor.matmul(ps_g[:, lb, :], m_all[:, bi, 0, :],
                                     m_all[:, bi, 1, :],
                                     start=False, stop=True)
                # masked max of raw Gram; tanh applied after the max
                # (tanh is monotone, so max commutes with it); latency of this
                # fused chain is hidden by the 3-supertile pooled distance
                cm = cm_pool.tile([128, 4], dt.float32, tag="cm", name=f"cm{s}")
                nc.vector.tensor_reduce(
                    out=cm[:, :w], in_=ps_g[:, :w, :],
                    op=mybir.AluOpType.max, axis=mybir.AxisListType.X)
                th = cm_pool.tile([128, 4], dt.float32, tag="th", name=f"th{s}")
                nc.scalar.activation(th[:, :w], cm[:, :w], AF.Tanh)
                ex = cm_pool.tile([128, 4], dt.float32, tag="ex", name=f"ex{s}")
                nc.scalar.activation(ex[:, :w], th[:, :w], AF.Exp)
                nunits = (w // 2) + (w % 2)
                # last supertiles' att on DVE: Pool's serial backlog would
                # otherwise gate the final pooled accumulations
                att_eng = nc.vector if s >= NST - 2 else nc.gpsimd
                for ui in range(nunits):
                    lb0 = 2 * ui
                    nb = 2 if lb0 + 1 < w else 1
                    u = unit_base[s] + ui
                    att_t = att_tiles[u]
                    halves = sorted(unit_halves[u])
                    io_sl = (slice(halves[0] * 128, (halves[0] + 1) * 128)
                             if len(halves) == 1 else slice(0, SLOTS))
                    for j in range(nb):
                        bi = st_off[s] + lb0 + j
                        att_eng.tensor_scalar(
                            out=att_t[:, j, :], in0=iota_f[:, io_sl],
                            scalar1=slot_all[:, bi:bi + 1],
                            scalar2=ex[:, lb0 + j:lb0 + j + 1],
                            op0=mybir.AluOpType.is_equal,
                            op1=mybir.AluOpType.mult)

            ps_p = [ps_pooled.tile([128, DW], dt.float32, tag=f"psp{h}",
                                   name=f"psp{h}") for h in range(2)]

            def emit_pooled(s):
                w = widths[s]
                xv = x_tiles[s]
                nunits = (w // 2) + (w % 2)
                for ui in range(nunits):
                    u = unit_base[s] + ui
                    lb0 = 2 * ui
                    nb = units[u][2]
                    att_t = att_tiles[u]
                    halves = sorted(unit_halves[u])
                    kw = ({"perf_mode": mybir.MatmulPerfMode.DoubleRow}
                          if nb == 2 else {})
                    for h in halves:
                        hsl = (slice(0, 128) if len(halves) == 1
                               else slice(h * 128, (h + 1) * 128))
                        # keep each matmul output inside one PSUM bank
                        for n0, nlen in ((0, 512), (512, DW - 512)):
                            nc.tensor.matmul(
                                ps_p[h][:, n0:n0 + nlen],
                                att_t[:, :, hsl],
                                xv[:, lb0:lb0 + nb, n0:n0 + nlen],
                                start=(u == chain[h][0]),
                                stop=(u == chain[h][-1]), **kw)

            for s in range(NST):
                emit_xw(s)
                if s + 2 < NST:
                    load_xt(s + 2)
                load_x(s)
                if s >= 1:
                    emit_bins(s - 1)
                if s >= 3:
                    emit_pooled(s - 3)
            emit_bins(NST - 1)
            emit_pooled(NST - 3)
            emit_pooled(NST - 2)
            emit_pooled(NST - 1)

            # FFN weights loaded late so they don't delay the phase-A DMA stream
            nc.sync.dma_start(out=w1t[:, :KC, :],
                              in_=w1t_d[:, :].rearrange("p (k h) -> p k h", k=KC))
            nc.sync.dma_start(out=w1t[0:1, KC, :], in_=w1b_d[:, :])
            w2t = consts.tile([128, HC], dt.float8e4)
            nc.sync.dma_start(out=w2t, in_=w2t_d[:, :])

            # ---- phase B1: normalize pooled by the accumulated denominator
            # (separate tiles per slot-half so each half's transposes only
            # wait on its own normalize)
            pooled_sb = [ffn_pool.tile([128, D], dt.bfloat16, tag=f"pooled{h}",
                                       name=f"pooled{h}") for h in range(2)]
            for h in range(2):
                rz = small.tile([128, 1], dt.float32, tag="rz", name=f"rz{h}")
                nc.vector.reciprocal(rz, ps_p[h][:, D:D + 1])
                if h == 0:
                    nc.scalar.activation(pooled_sb[h], ps_p[h][:, :D],
                                         AF.Copy, scale=rz)
                else:
                    nc.vector.tensor_scalar_mul(pooled_sb[h],
                                                ps_p[h][:, :D], rz)

        # ---- phase B2: transpose pooled, FFN, hinge loss
        with (
            tc.tile_pool(name="ps_t", bufs=3, space="PSUM") as ps_t,
            tc.tile_pool(name="ps_h", bufs=3, space="PSUM") as ps_h,
            tc.tile_pool(name="ps_sc", bufs=1, space="PSUM") as ps_sc,
        ):
            # transpose pooled in batches of 4: one [128, 4*128] PSUM tile per
            # eviction so the PSUM->SBUF copies are few and wide. pooledT has
            # 2 extra contraction chunks: chunk KC carries a ones row at
            # partition 0 (pairs with the b1 row folded into w1t on host) and
            # chunk KC+1 is zero padding for the DoubleRow pair.
            pooledT = ffn_pool.tile([128, KC + 2, SLOTS], dt.float8e4,
                                    tag="pooledT")
            nc.gpsimd.memset(pooledT[:, KC:KC + 2, :], 0.0)
            nc.gpsimd.memset(pooledT[0:1, KC, :], 1.0)
            # h-major so the first transposes only wait on h0's normalize
            tr_jobs = [(k, h) for h in range(2) for k in range(KC)]
            for grp in range(3):
                ps_tr = ps_t.tile([128, 4, 128], dt.bfloat16, tag="pstr",
                                  name=f"pstr{grp}")
                for q in range(4):
                    k, h = tr_jobs[4 * grp + q]
                    nc.tensor.transpose(
                        ps_tr[:, q, :], pooled_sb[h][:, k * 128:(k + 1) * 128],
                        ident)
                # each half is (k, h), (k+1, h): a strided [128, 2, 128] span
                for half in range(2):
                    k0, h0 = tr_jobs[4 * grp + 2 * half]
                    src = ps_tr[:, 2 * half:2 * half + 2, :]
                    dst = pooledT[:, k0:k0 + 2, h0 * 128:(h0 + 1) * 128]
                    if (grp + half) % 2 == 0:
                        nc.scalar.copy(dst, src)
                    else:
                        nc.vector.tensor_copy(dst, src)
            # small accumulators share one PSUM bank
            sm_ps = ps_sc.tile([128, 8], dt.float32, tag="sm", name="sm_ps")
            ps_sT = sm_ps[:, 0:2]
            # scores accumulate directly in slot-partition form: stationary =
            # hrelu pair slices (wide, satisfies fp8 dual-row Ldweights rules),
            # moving = w2 pairs -> out [slot, 1] per chunk-half. hrelu lives in
            # per-pair tiles so the score matmuls don't serialize against
            # later hrelu writes (dependencies are tile-granular).
            for j in range(HC // 2):
                ps_hh = ps_h.tile([128, 2, SLOTS], dt.float32, tag="psh",
                                  name=f"psh{j}")
                hrelu = ffn_pool.tile([128, 2, SLOTS], dt.float8e4,
                                      tag=f"hrelu{j}")
                for q in range(2):
                    hc = 2 * j + q
                    for k in range(0, KC + 2, 2):
                        nc.tensor.matmul(ps_hh[:, q, :],
                                         w1t[:, k:k + 2, hc * 128:(hc + 1) * 128],
                                         pooledT[:, k:k + 2, :],
                                         start=(k == 0), stop=(k == KC),
                                         perf_mode=mybir.MatmulPerfMode.DoubleRow)
                # W1,b1 host-scaled by 8 (b1 folded into w1t's ones-chunk row):
                # hrelu holds 8*h; 1/8 folded into the sigmoid scale below.
                # one wide eviction per pair, alternating engines
                if j % 2 == 0:
                    nc.scalar.activation(hrelu, ps_hh, AF.Relu)
                else:
                    nc.vector.tensor_scalar_max(hrelu, ps_hh, 0.0)
                w2p = w2t[:, 2 * j:2 * j + 2].rearrange("p (k o) -> p k o", o=1)
                for ch in range(2):
                    nc.tensor.matmul(
                        ps_sT[:, ch:ch + 1],
                        hrelu[:, :, ch * 128:(ch + 1) * 128],
                        w2p,
                        start=(j == 0), stop=(j == HC // 2 - 1),
                        perf_mode=mybir.MatmulPerfMode.DoubleRow)
            # raw slot scores out; sigmoid + hinge run on host (trivial
            # scalar work, off the device critical path)
            sT = ffn_pool.tile([128, 2], dt.float32, tag="sT")
            nc.vector.tensor_copy(sT, ps_sT)
            nc.sync.dma_start(out=loss_d[:, :], in_=sT)

    _split_waits(nc)
    return nc


# ---------------------------------------------------------------- entry point

def kernel(triple_emb, W_sfa, W1, b1, W2, b2, tri2path_size):
    _patch_tile_drain()
    triple_emb = np.asarray(triple_emb, np.float32)
    sizes_flat = np.asarray(tri2path_size, np.int32).reshape(-1).astype(np.int64)
    offsets = np.concatenate([[0], np.cumsum(sizes_flat)[:-1]])

    core_rows, bins_all = _pack(sizes_flat)
    NB = max(len(b) for b in bins_all)
    MG = max(max(len(bn) for bn in bins) for bins in bins_all) + 1
    MG = ((MG + 7) // 8) * 8
    MG = max(MG, 40)

    triple_f8 = triple_emb.astype(bf16).astype(fp8e4)
    wsfa_t = np.ascontiguousarray(
        np.asarray(W_sfa, np.float32).T.reshape(KC, 128, D).transpose(1, 0, 2)
        .reshape(128, KC * D)).astype(fp8e4)
    w1_t = np.ascontiguousarray(
        (np.asarray(W1, np.float32) * 8.0).T.reshape(KC, 128, 4 * D)
        .transpose(1, 0, 2).reshape(128, KC * 4 * D)).astype(fp8e4)
    w2_t = np.ascontiguousarray(
        (np.asarray(W2, np.float32) * 16.0).reshape(HC, 128).T).astype(fp8e4)
    w1_b = (np.asarray(b1, np.float32) * 8.0).reshape(1, 4 * D).astype(fp8e4)

    in_maps = []
    for c in range(NCORES):
        # pad this core's bin list to NB bins (empty bins are all-masked)
        bins_c = bins_all[c] + [[]] * (NB - len(bins_all[c]))
        x, xt_flat, m, slot_of = _build_core_arrays(
            bins_c, triple_f8, offsets, NB, MG)
        in_maps.append({
            "x_bins": x, "xt_bins": xt_flat, "mask_f": m, "slot_of": slot_of,
            "w_sfa_t": wsfa_t, "w1_t": w1_t, "w2_t": w2_t,
            "w1_b": w1_b,
        })

    widths = _widths_of(NB)
    nunits_tot = sum((w // 2) + (w % 2) for w in widths)
    unit_halves = tuple(frozenset({0, 1}) for _ in range(nunits_tot))

    with _compile_lock:
        key = (widths, MG, unit_halves)
        nc = _compile_cache.get(key)
        if nc is None:
            nc = _build_program(widths, MG, unit_halves)
            _compile_cache[key] = nc

    res = run_bass_kernel_spmd(nc, in_maps, core_ids=list(range(NCORES)),
                               trace=bool(int(os.environ.get("KGE_TRACE", "0"))))
    # host tail: sigmoid (W2 x16, W1 x8 host scalings -> /128) + hinge sum
    b2f = np.float64(np.asarray(b2, np.float32).reshape(())) if np.asarray(b2).size         else np.float64(0.0)
    total = np.float64(0.0)
    for r in res.results:
        raw = np.asarray(r["scores"], np.float64)          # [128, 2]
        sc = 1.0 / (1.0 + np.exp(-(raw.T.reshape(-1) / 128.0 + b2f)))
        sc = sc.reshape(ROWS_PER_CORE, NEG + 1)
        total += np.maximum(sc[:, 1:] + GAMMA - sc[:, :1], 0.0).sum()
    kernel.last_results = res
    return np.asarray(np.float32(total))
